# revision 9
# baseline (speedup 1.0000x reference)
"""Trainium2 Bass kernel for nn_BlockSelfAttentionModule (v6, bf16, software-pipelined).

Same math as v5. v6 restructures the rearrangement stages to minimize
instruction count and rebalance engines:

  stage 0 load : q DMA (sync)                          -> qsb
  stage 1 mm   : 4 bf16 matmuls (PE) + 4 PSUM->SBUF casts (ACT) -> gsb
  stage 2 skew : HBM round-trip skew (2 DMAs on one scalar HWDGE queue):
                 store g-windows gsb->gscr (DRAM), load back with the
                 per-partition-group diagonal on the DRAM-side AP -> ts12
                 (replaces 12 strided DMAs + 4 DVE copies of v5; SBUF-side
                 diagonal APs are illegal, DRAM-side ones are not)
                 8 DVE copy_predicated voice-select ops -> vt12
  stage 3 out  : broadcast adds (DVE: b0,b1 merged + b3 c1/c2;
                 POOL: b2 merged + b3 c0) + 1 merged store (sync)

Per-iteration dma_starts: 4 (q load, skew out/in, store) vs 13 in v5.
Skew store_i+1 cannot pass load_i: both sit on the scalar HWDGE FIFO.
"""

import os
import sys

for _p in ("/opt/trn_rl_repo", "/root/.axon_site/_ro/trn_rl_repo"):
    if os.path.isdir(_p) and _p not in sys.path:
        sys.path.insert(0, _p)

import contextlib

import numpy as np
import ml_dtypes

import concourse.bass as bass
import concourse.bacc as bacc
import concourse.mybir as mybir
import concourse.tile as tile
from concourse.bass_utils import run_bass_kernel_spmd

E, H, DI, DO, F = 16, 8, 8, 8, 48
L = F * DI  # 384
B = 4
NJ = 3 * B  # 12
GP = 1536
TP = NJ * F  # 576
VP = NJ * 9  # 108 (9-padded per bc so copy_predicated APs stay unmerged)
NCORES = 8
BF = mybir.dt.bfloat16
F32 = mybir.dt.float32

_prog_cache = {}

U8 = mybir.dt.uint8
MSK = np.ascontiguousarray(
    (np.arange(128)[:, None] % 8 == np.arange(8)[None, :]).astype(np.uint8)
)


def build_program(loop_n=None, unroll=4, cfg=None):
    cfg = cfg or {}
    # engine for each of the 8 voice-select copies (indexed by di)
    voice_eng = cfg.get("voice_eng", "scalar")
    # add plan: list of (b, chunks, engine) where chunks is a tuple of c's
    # done in one merged op (or single-c tuples)
    add_plan = cfg.get(
        "add_plan",
        [
            (0, (0, 1, 2), "vector"),
            (1, (0, 1, 2), "vector"),
            (2, (0, 1, 2), "gpsimd"),
            (3, (0,), "gpsimd"),
            (3, (1,), "vector"),
            (3, (2,), "gpsimd"),
        ],
    )
    zcopy_eng = cfg.get("zcopy_eng", ("scalar",) * 4)
    trip = loop_n if loop_n is not None else 1

    nc = bacc.Bacc("TRN2", target_bir_lowering=False, debug=False)
    winp = nc.dram_tensor("winp", [48, 384], BF, kind="ExternalInput")
    qinp = nc.dram_tensor("qinp", [48, 512], BF, kind="ExternalInput")
    mk = nc.dram_tensor("mk", [128, 8], U8, kind="ExternalInput")
    out = nc.dram_tensor("out", [B, L, L], BF, kind="ExternalOutput")
    # skew scratch: gscr[p, bc, w] = gsb[p, 128*bc + w], w in [0, 63)
    GW = 63  # g-window width per (partition, bc)
    GSP = NJ * GW  # 756 elements per partition row in scratch
    gscr = nc.dram_tensor("gscr", [128, GSP], BF, kind="Internal")

    with tile.TileContext(nc) as tc, contextlib.ExitStack() as ctx:
        const_pool = ctx.enter_context(tc.tile_pool(name="const", bufs=1))
        zp_pool = ctx.enter_context(tc.tile_pool(name="zp", bufs=8, space="PSUM"))
        osb_pool = ctx.enter_context(tc.tile_pool(name="osb", bufs=2))

        wsb = const_pool.tile([48, 384], BF)
        msk = const_pool.tile([128, 8], U8)
        nc.scalar.dma_start(wsb[:], winp[:])
        nc.sync.dma_start(msk[:], mk[:])

        def eng(name):
            return getattr(nc, name)

        def stage_load(pipe, iv):
            qsb = pipe.intermediate_tile([48, 512], BF, name="qsb")
            nc.sync.dma_start(qsb[:], qinp[:])
            return qsb

        def stage_mm(pipe, iv, qsb):
            gsb = pipe.intermediate_tile([128, GP], BF, name="gsb")
            z_tiles = []
            for b in range(B):
                lhsT = bass.AP(qsb.tensor, 128 * b, [[512, 48], [1, 128]])
                z = zp_pool.tile([128, 384], F32, tag="z")
                nc.tensor.matmul(z[:], lhsT, wsb[:])
                z_tiles.append(z)
            for b in range(B):
                dst = bass.AP(gsb.tensor, 384 * b, [[GP, 128], [1, 384]])
                eng(zcopy_eng[b]).copy(dst, z_tiles[b][:])
            return gsb

        def stage_skew(pipe, iv, gsb):
            ts12 = pipe.intermediate_tile([128, TP], BF, name="ts12")
            vt12 = pipe.intermediate_tile([128, VP], BF, name="vt12")
            # skew via HBM round-trip (both DMAs on the scalar HWDGE FIFO so
            # next iteration's store cannot pass this iteration's load).
            # out: gscr[p][bc][w] = gsb[p, 128*bc + w], w in [0, GW)
            so_src = bass.AP(gsb.tensor, 0, [[GP, 128], [128, NJ], [1, GW]])
            so_dst = bass.AP(gscr, 0, [[GSP, 128], [GW, NJ], [1, GW]])
            nc.scalar.dma_start(so_dst, so_src)
            # in: ts12[8fp+v, 48*bc + s] = gscr[8fp+v][bc][fp + s]
            # DRAM-side diagonal: fp advances 8 partition rows AND +1 element.
            si_src = bass.AP(
                gscr, 0, [[8 * GSP + 1, 16], [GSP, 8], [GW, NJ], [1, F]]
            )
            si_dst = bass.AP(ts12.tensor, 0, [[TP, 128], [F, NJ], [1, F]])
            nc.scalar.dma_start(si_dst, si_src)
            # voice select: partition j keeps di == j%8: vt12 col 8*bc + do
            for di in range(8):
                data = bass.AP(
                    gsb.tensor, 64 + 8 * di,
                    [[GP, 128], [128, NJ], [1, 8]],
                )
                mask = bass.AP(msk.tensor, di, [[8, 128], [0, NJ], [0, 8]])
                vout = bass.AP(
                    vt12.tensor, 0, [[VP, 128], [9, NJ], [1, 8]]
                )
                nc.vector.copy_predicated(vout, mask, data)
            return (ts12, vt12)

        def stage_out(pipe, iv, tv):
            ts12, vt12 = tv
            osb = osb_pool.tile([128, B * 3 * L], BF, tag="osb")
            for b, chunks, ename in add_plan:
                c0 = chunks[0]
                nch = len(chunks)
                j0 = 3 * b + c0
                if nch > 1:
                    t_b = bass.AP(
                        ts12.tensor, j0 * F + 47,
                        [[TP, 128], [F, nch], [-1, F], [0, 8]],
                    )
                    v_b = bass.AP(
                        vt12.tensor, j0 * 9,
                        [[VP, 128], [9, nch], [0, F], [1, 8]],
                    )
                    o_ap = bass.AP(
                        osb.tensor, j0 * L,
                        [[B * 3 * L, 128], [L, nch], [8, F], [1, 8]],
                    )
                else:
                    t_b = bass.AP(
                        ts12.tensor, j0 * F + 47, [[TP, 128], [-1, F], [0, 8]]
                    )
                    v_b = bass.AP(
                        vt12.tensor, j0 * 9, [[VP, 128], [0, F], [1, 8]]
                    )
                    o_ap = bass.AP(
                        osb.tensor, j0 * L, [[B * 3 * L, 128], [8, F], [1, 8]]
                    )
                eng(ename).tensor_add(o_ap, t_b, v_b)
            st_src = bass.AP(osb.tensor, 0, [[B * 3 * L, 128], [L, 12], [1, L]])
            st_dst = bass.AP(out, 0, [[L, 128], [128 * L, 12], [1, L]])
            nc.sync.dma_start(st_dst, st_src)

        tc.For_i_pipelined(
            [stage_load, stage_mm, stage_skew, stage_out],
            0,
            trip,
            unroll=unroll,
        )

    nc.compile()
    return nc


def _get_program():
    if "nc" not in _prog_cache:
        _prog_cache["nc"] = build_program()
    return _prog_cache["nc"]


def make_core_inputs(q, r_voice, e_past, e_future):
    q = np.asarray(q, dtype=np.float32)
    qb = q.reshape(B, H, L, E)
    in_maps = []
    for h in range(NCORES):
        w = np.zeros((48, 384), np.float32)
        gfull = np.empty((E, 95), np.float32)
        gfull[:, 0:47] = e_future[1:48, :, h][::-1].T
        gfull[:, 47:95] = e_past[:, :, h].T
        u = r_voice[:, :, :, h].reshape(DI * DO, E).T  # (E, 64)
        for c in range(3):
            blk = np.zeros((E, 128), np.float32)
            blk[:, 0:63] = gfull[:, 16 * c:16 * c + 63]
            blk[:, 64:128] = u
            w[c * 16:(c + 1) * 16, 128 * c:128 * (c + 1)] = blk
        qh = qb[:, h]  # (B, L, E)
        qt = qh.reshape(B, 3, 128, E).transpose(1, 3, 0, 2).reshape(48, 512)
        in_maps.append(
            {
                "winp": w.astype(ml_dtypes.bfloat16),
                "qinp": np.ascontiguousarray(qt).astype(ml_dtypes.bfloat16),
                "mk": MSK,
            }
        )
    return in_maps


def kernel(q, flipped_masks, r_voice, e_past, e_future):
    q = np.asarray(q, dtype=np.float32)
    r_voice = np.asarray(r_voice, dtype=np.float32)
    e_past = np.asarray(e_past, dtype=np.float32)
    e_future = np.asarray(e_future, dtype=np.float32)

    nc = _get_program()
    in_maps = make_core_inputs(q, r_voice, e_past, e_future)
    res = run_bass_kernel_spmd(nc, in_maps, core_ids=list(range(NCORES)))

    out = np.empty((B * H, L, L), dtype=np.float32)
    for h in range(NCORES):
        out_h = np.asarray(res.results[h]["out"]).astype(np.float32)
        for b in range(B):
            out[b * H + h] = out_h[b]
    return out


# revision 15
# speedup vs baseline: 1.7156x; 1.7156x over previous
"""Trainium2 Bass kernel for nn_BlockSelfAttentionModule (v7, bf16, software-pipelined).

Structure (per iteration):
  stage 0 load : q DMA (sync)                                   -> qsb
  stage 1 mm   : 4 bf16 matmuls (PE) + 4 PSUM->SBUF casts (ACT) -> gsb
  stage 2 skew : 16 per-partition-group skew ops split across
                 scalar-HWDGE DMAs / ACT copies / DVE copies    -> ts12
                 1 merged DVE copy_predicated voice select      -> vt12
  stage 3 out  : 12 (b,c)-tile broadcast adds split DVE/POOL    -> osb
                 1 contiguous store (sync)                      -> out

HW findings baked in (probed on device):
  - output stored contiguously per partition ([128, 4608], 9216B
    descriptors) is 3.8us/iter vs 6.2us for the [B, L, L] scattered
    layout; the host unshard absorbs the permutation.
  - splitting the contiguous store across queues makes it slower.
  - HBM round-trip skew (2 DMAs) is ~4us slower than 16 split ops.
  - SBUF-side APs cannot mix partition and column strides, so the skew
    is inherently 16 ops (one per partition-group col offset).
"""

import os
import sys

for _p in ("/opt/trn_rl_repo", "/root/.axon_site/_ro/trn_rl_repo"):
    if os.path.isdir(_p) and _p not in sys.path:
        sys.path.insert(0, _p)

import contextlib

import numpy as np
import ml_dtypes

import concourse.bass as bass
import concourse.bacc as bacc
import concourse.mybir as mybir
import concourse.tile as tile
from concourse.bass_utils import run_bass_kernel_spmd

E, H, DI, DO, F = 16, 8, 8, 8, 48
L = F * DI  # 384
B = 4
NJ = 3 * B  # 12
GP = 1536
TP = NJ * F  # 576
VP = NJ * 9  # 108 (9-padded per bc so copy_predicated APs stay unmerged)
NCORES = 8
BF = mybir.dt.bfloat16
F32 = mybir.dt.float32
U8 = mybir.dt.uint8

_prog_cache = {}

MSK = np.ascontiguousarray(
    (np.arange(128)[:, None] % 8 == np.arange(8)[None, :]).astype(np.uint8)
)

# skew: op kind per partition-group fp. "dmaS"/"dmaQ"/"dmaG" = DMA on
# scalar/sync/gpsimd queue; "act"/"vec"/"pool" = engine copies.
# engine copies only legal at fp 0/4/8/12 (32-aligned base partition)
SKEW_PLAN = (
    "vec", "dmaS", "dmaS", "dmaG", "act", "dmaS", "dmaS", "dmaG",
    "vec", "dmaS", "dmaS", "dmaG", "act", "dmaS", "dmaS", "dmaG",
)
# adds: (b, chunks, engine)
ADD_PLAN = (
    (0, (0, 1, 2), "vector"),
    (1, (0, 1, 2), "vector"),
    (2, (0, 1, 2), "gpsimd"),
    (3, (0,), "gpsimd"),
    (3, (1,), "vector"),
    (3, (2,), "gpsimd"),
)


def build_program(loop_n=None, unroll=4, cfg=None):
    cfg = cfg or {}
    skew_plan = cfg.get("skew_plan", SKEW_PLAN)
    add_plan = cfg.get("add_plan", ADD_PLAN)
    zcopy_eng = cfg.get("zcopy_eng", ("scalar",) * 4)
    trip = loop_n if loop_n is not None else 1

    nc = bacc.Bacc("TRN2", target_bir_lowering=False, debug=False)
    winp = nc.dram_tensor("winp", [48, 384], BF, kind="ExternalInput")
    qinp = nc.dram_tensor("qinp", [48, 512], BF, kind="ExternalInput")
    mk = nc.dram_tensor("mk", [128, 8], U8, kind="ExternalInput")
    out = nc.dram_tensor("out", [128, NJ * L], BF, kind="ExternalOutput")

    with tile.TileContext(nc) as tc, contextlib.ExitStack() as ctx:
        const_pool = ctx.enter_context(tc.tile_pool(name="const", bufs=1))
        zp_pool = ctx.enter_context(tc.tile_pool(name="zp", bufs=8, space="PSUM"))
        osb_pool = ctx.enter_context(tc.tile_pool(name="osb", bufs=2))

        wsb = const_pool.tile([48, 384], BF)
        msk = const_pool.tile([128, 8], U8)
        nc.scalar.dma_start(wsb[:], winp[:])
        nc.sync.dma_start(msk[:], mk[:])

        def eng(name):
            return getattr(nc, name)

        def stage_load(pipe, iv):
            qsb = pipe.intermediate_tile([48, 512], BF, name="qsb")
            if not cfg.get("no_load"):
                nc.sync.dma_start(qsb[:], qinp[:])
            return qsb

        def stage_mm(pipe, iv, qsb):
            gsb = pipe.intermediate_tile([128, GP], BF, name="gsb")
            if cfg.get("no_mm"):
                nc.vector.memset(gsb[:, 0:8], 0)
                return gsb
            z_tiles = []
            for b in range(B):
                lhsT = bass.AP(qsb.tensor, 128 * b, [[512, 48], [1, 128]])
                z = zp_pool.tile([128, 384], F32, tag="z")
                nc.tensor.matmul(z[:], lhsT, wsb[:])
                z_tiles.append(z)
            for b in range(B):
                dst = bass.AP(gsb.tensor, 384 * b, [[GP, 128], [1, 384]])
                eng(zcopy_eng[b]).copy(dst, z_tiles[b][:])
            return gsb

        def stage_skew(pipe, iv, gsb):
            ts12 = pipe.intermediate_tile([128, TP], BF, name="ts12")
            vt12 = pipe.intermediate_tile([128, VP], BF, name="vt12")
            if not skew_plan:
                nc.vector.memset(ts12[:, 0:8], 0)
            # skew: group fp (partitions 8fp..8fp+8) reads gsb cols
            # [128*bc + fp, +48) into ts12 cols [48*bc, +48).
            for fp, kind in enumerate(skew_plan):
                s_ap = bass.AP(
                    gsb.tensor, 8 * fp * GP + fp, [[GP, 8], [128, NJ], [1, F]]
                )
                d_ap = bass.AP(
                    ts12.tensor, 8 * fp * TP, [[TP, 8], [F, NJ], [1, F]]
                )
                if kind == "dmaS":
                    nc.scalar.dma_start(d_ap, s_ap)
                elif kind == "dmaQ":
                    nc.sync.dma_start(d_ap, s_ap)
                elif kind == "dmaG":
                    nc.gpsimd.dma_start(d_ap, s_ap)
                elif kind == "act":
                    nc.scalar.copy(d_ap, s_ap)
                elif kind == "vec":
                    nc.vector.tensor_copy(d_ap, s_ap)
                else:
                    assert kind == "pool"
                    nc.gpsimd.tensor_copy(d_ap, s_ap)
            if cfg.get("no_voice"):
                nc.vector.memset(vt12[:, 0:8], 0)
                return (ts12, vt12)
            # voice select: vt12[p, 9*bc + do] = gsb[p, 128*bc+64+8*(p%8)+do]
            for di in range(8):
                data = bass.AP(
                    gsb.tensor, 64 + 8 * di, [[GP, 128], [128, NJ], [1, 8]]
                )
                mask = bass.AP(msk.tensor, di, [[8, 128], [0, NJ], [0, 8]])
                vout = bass.AP(
                    vt12.tensor, 0, [[VP, 128], [9, NJ], [1, 8]]
                )
                nc.vector.copy_predicated(vout, mask, data)
            return (ts12, vt12)

        def stage_out(pipe, iv, tv):
            ts12, vt12 = tv
            osb = osb_pool.tile([128, B * 3 * L], BF, tag="osb")
            if cfg.get("no_adds"):
                nc.vector.memset(osb[:, 0:8], 0)
            for b, chunks, ename in ([] if cfg.get("no_adds") else add_plan):
                c0 = chunks[0]
                nch = len(chunks)
                j0 = 3 * b + c0
                if nch > 1:
                    t_b = bass.AP(
                        ts12.tensor, j0 * F + 47,
                        [[TP, 128], [F, nch], [-1, F], [0, 8]],
                    )
                    v_b = bass.AP(
                        vt12.tensor, j0 * 9,
                        [[VP, 128], [9, nch], [0, F], [1, 8]],
                    )
                    o_ap = bass.AP(
                        osb.tensor, j0 * L,
                        [[B * 3 * L, 128], [L, nch], [8, F], [1, 8]],
                    )
                else:
                    t_b = bass.AP(
                        ts12.tensor, j0 * F + 47, [[TP, 128], [-1, F], [0, 8]]
                    )
                    v_b = bass.AP(
                        vt12.tensor, j0 * 9, [[VP, 128], [0, F], [1, 8]]
                    )
                    o_ap = bass.AP(
                        osb.tensor, j0 * L, [[B * 3 * L, 128], [8, F], [1, 8]]
                    )
                eng(ename).tensor_add(o_ap, t_b, v_b)
            if cfg.get("no_store"):
                return
            st_src = bass.AP(osb.tensor, 0, [[B * 3 * L, 128], [1, NJ * L]])
            st_dst = bass.AP(out, 0, [[NJ * L, 128], [1, NJ * L]])
            nc.sync.dma_start(st_dst, st_src)

        tc.For_i_pipelined(
            [stage_load, stage_mm, stage_skew, stage_out],
            0,
            trip,
            unroll=unroll,
        )

    nc.compile()
    return nc


def _get_program():
    if "nc" not in _prog_cache:
        _prog_cache["nc"] = build_program()
    return _prog_cache["nc"]


def make_core_inputs(q, r_voice, e_past, e_future):
    q = np.asarray(q, dtype=np.float32)
    qb = q.reshape(B, H, L, E)
    in_maps = []
    for h in range(NCORES):
        w = np.zeros((48, 384), np.float32)
        gfull = np.empty((E, 95), np.float32)
        gfull[:, 0:47] = e_future[1:48, :, h][::-1].T
        gfull[:, 47:95] = e_past[:, :, h].T
        u = r_voice[:, :, :, h].reshape(DI * DO, E).T  # (E, 64)
        for c in range(3):
            blk = np.zeros((E, 128), np.float32)
            blk[:, 0:63] = gfull[:, 16 * c:16 * c + 63]
            blk[:, 64:128] = u
            w[c * 16:(c + 1) * 16, 128 * c:128 * (c + 1)] = blk
        qh = qb[:, h]  # (B, L, E)
        qt = qh.reshape(B, 3, 128, E).transpose(1, 3, 0, 2).reshape(48, 512)
        in_maps.append(
            {
                "winp": w.astype(ml_dtypes.bfloat16),
                "qinp": np.ascontiguousarray(qt).astype(ml_dtypes.bfloat16),
                "mk": MSK,
            }
        )
    return in_maps


def unshard(out_h):
    """out_h: [128, NJ*L] device layout -> [B, L, L] float32."""
    arr = np.asarray(out_h).astype(np.float32).reshape(128, B, 3, L)
    return arr.transpose(1, 2, 0, 3).reshape(B, L, L)


def kernel(q, flipped_masks, r_voice, e_past, e_future):
    q = np.asarray(q, dtype=np.float32)
    r_voice = np.asarray(r_voice, dtype=np.float32)
    e_past = np.asarray(e_past, dtype=np.float32)
    e_future = np.asarray(e_future, dtype=np.float32)

    nc = _get_program()
    in_maps = make_core_inputs(q, r_voice, e_past, e_future)
    res = run_bass_kernel_spmd(nc, in_maps, core_ids=list(range(NCORES)))

    out = np.empty((B * H, L, L), dtype=np.float32)
    for h in range(NCORES):
        out_b = unshard(res.results[h]["out"])
        for b in range(B):
            out[b * H + h] = out_b[b]
    return out


# revision 18
# speedup vs baseline: 1.7630x; 1.0276x over previous
"""Trainium2 Bass kernel for nn_BlockSelfAttentionModule (v7, bf16, software-pipelined).

Structure (per iteration):
  stage 0 load : q DMA (sync)                                   -> qsb
  stage 1 mm   : 4 bf16 matmuls (PE) + 4 PSUM->SBUF casts (ACT) -> gsb
  stage 2 skew : 16 per-partition-group skew ops split across
                 scalar-HWDGE DMAs / ACT copies / DVE copies    -> ts12
                 1 merged DVE copy_predicated voice select      -> vt12
  stage 3 out  : 12 (b,c)-tile broadcast adds split DVE/POOL    -> osb
                 1 contiguous store (sync)                      -> out

HW findings baked in (probed on device):
  - output stored contiguously per partition ([128, 4608], 9216B
    descriptors) is 3.8us/iter vs 6.2us for the [B, L, L] scattered
    layout; the host unshard absorbs the permutation.
  - splitting the contiguous store across queues makes it slower.
  - HBM round-trip skew (2 DMAs) is ~4us slower than 16 split ops.
  - SBUF-side APs cannot mix partition and column strides, so the skew
    is inherently 16 ops (one per partition-group col offset).
"""

import os
import sys

for _p in ("/opt/trn_rl_repo", "/root/.axon_site/_ro/trn_rl_repo"):
    if os.path.isdir(_p) and _p not in sys.path:
        sys.path.insert(0, _p)

import contextlib

import numpy as np
import ml_dtypes

import concourse.bass as bass
import concourse.bacc as bacc
import concourse.mybir as mybir
import concourse.tile as tile
from concourse.bass_utils import run_bass_kernel_spmd

E, H, DI, DO, F = 16, 8, 8, 8, 48
L = F * DI  # 384
B = 4
NJ = 3 * B  # 12
GP = 1536
TP = NJ * F  # 576
VP = NJ * 9  # 108 (9-padded per bc so copy_predicated APs stay unmerged)
NCORES = 8
BF = mybir.dt.bfloat16
F32 = mybir.dt.float32
U8 = mybir.dt.uint8

_prog_cache = {}

MSK = np.ascontiguousarray(
    (np.arange(128)[:, None] % 8 == np.arange(8)[None, :]).astype(np.uint8)
)

# skew: op kind per partition-group fp. "dmaS"/"dmaQ"/"dmaG" = DMA on
# scalar/sync/gpsimd queue; "act"/"vec"/"pool" = engine copies.
# engine copies only legal at fp 0/4/8/12 (32-aligned base partition)
SKEW_PLAN = (
    "vec", "dmaS", "dmaS", "dmaG", "act", "dmaS", "dmaS", "dmaG",
    "vec", "dmaS", "dmaS", "dmaG", "act", "dmaS", "dmaS", "dmaG",
)
# adds: (b, chunks, engine)
ADD_PLAN = (
    (0, (0, 1, 2), "vector"),
    (1, (0, 1, 2), "vector"),
    (2, (0, 1, 2), "gpsimd"),
    (3, (0,), "gpsimd"),
    (3, (1,), "vector"),
    (3, (2,), "vector"),
)


def build_program(loop_n=None, unroll=6, cfg=None):
    cfg = cfg or {}
    unroll = cfg.get("unroll", unroll)
    skew_plan = cfg.get("skew_plan", SKEW_PLAN)
    add_plan = cfg.get("add_plan", ADD_PLAN)
    zcopy_eng = cfg.get("zcopy_eng", ("scalar",) * 4)
    trip = loop_n if loop_n is not None else 1

    nc = bacc.Bacc("TRN2", target_bir_lowering=False, debug=False)
    winp = nc.dram_tensor("winp", [48, 384], BF, kind="ExternalInput")
    qinp = nc.dram_tensor("qinp", [48, 512], BF, kind="ExternalInput")
    mk = nc.dram_tensor("mk", [128, 8], U8, kind="ExternalInput")
    out = nc.dram_tensor("out", [128, NJ * L], BF, kind="ExternalOutput")

    with tile.TileContext(nc) as tc, contextlib.ExitStack() as ctx:
        const_pool = ctx.enter_context(tc.tile_pool(name="const", bufs=1))
        zp_pool = ctx.enter_context(tc.tile_pool(name="zp", bufs=8, space="PSUM"))
        osb_pool = ctx.enter_context(tc.tile_pool(name="osb", bufs=2))

        wsb = const_pool.tile([48, 384], BF)
        msk = const_pool.tile([128, 8], U8)
        nc.scalar.dma_start(wsb[:], winp[:])
        nc.sync.dma_start(msk[:], mk[:])

        def eng(name):
            return getattr(nc, name)

        def stage_load(pipe, iv):
            qsb = pipe.intermediate_tile([48, 512], BF, name="qsb")
            if not cfg.get("no_load"):
                eng(cfg.get("q_eng", "scalar")).dma_start(qsb[:], qinp[:])
            return qsb

        def stage_mm(pipe, iv, qsb):
            gsb = pipe.intermediate_tile([128, GP], BF, name="gsb")
            if cfg.get("no_mm"):
                nc.vector.memset(gsb[:, 0:8], 0)
                return gsb
            z_tiles = []
            for b in range(B):
                lhsT = bass.AP(qsb.tensor, 128 * b, [[512, 48], [1, 128]])
                z = zp_pool.tile([128, 384], F32, tag="z")
                nc.tensor.matmul(z[:], lhsT, wsb[:])
                z_tiles.append(z)
            for b in range(B):
                dst = bass.AP(gsb.tensor, 384 * b, [[GP, 128], [1, 384]])
                if zcopy_eng[b] == "scalar":
                    nc.scalar.copy(dst, z_tiles[b][:])
                else:
                    eng(zcopy_eng[b]).tensor_copy(dst, z_tiles[b][:])
            return gsb

        def stage_skew(pipe, iv, gsb):
            ts12 = pipe.intermediate_tile([128, TP], BF, name="ts12")
            vt12 = pipe.intermediate_tile([128, VP], BF, name="vt12")
            if not skew_plan:
                nc.vector.memset(ts12[:, 0:8], 0)
            # skew: group fp (partitions 8fp..8fp+8) reads gsb cols
            # [128*bc + fp, +48) into ts12 cols [48*bc, +48).
            for fp, kind in enumerate(skew_plan):
                s_ap = bass.AP(
                    gsb.tensor, 8 * fp * GP + fp, [[GP, 8], [128, NJ], [1, F]]
                )
                d_ap = bass.AP(
                    ts12.tensor, 8 * fp * TP, [[TP, 8], [F, NJ], [1, F]]
                )
                if kind == "dmaS":
                    nc.scalar.dma_start(d_ap, s_ap)
                elif kind == "dmaQ":
                    nc.sync.dma_start(d_ap, s_ap)
                elif kind == "dmaG":
                    nc.gpsimd.dma_start(d_ap, s_ap)
                elif kind == "act":
                    nc.scalar.copy(d_ap, s_ap)
                elif kind == "vec":
                    nc.vector.tensor_copy(d_ap, s_ap)
                else:
                    assert kind == "pool"
                    nc.gpsimd.tensor_copy(d_ap, s_ap)
            if cfg.get("no_voice"):
                nc.vector.memset(vt12[:, 0:8], 0)
                return (ts12, vt12)
            # voice select: vt12[p, 9*bc + do] = gsb[p, 128*bc+64+8*(p%8)+do]
            for di in range(8):
                data = bass.AP(
                    gsb.tensor, 64 + 8 * di, [[GP, 128], [128, NJ], [1, 8]]
                )
                mask = bass.AP(msk.tensor, di, [[8, 128], [0, NJ], [0, 8]])
                vout = bass.AP(
                    vt12.tensor, 0, [[VP, 128], [9, NJ], [1, 8]]
                )
                nc.vector.copy_predicated(vout, mask, data)
            return (ts12, vt12)

        def stage_out(pipe, iv, tv):
            ts12, vt12 = tv
            osb = osb_pool.tile([128, B * 3 * L], BF, tag="osb")
            if cfg.get("no_adds"):
                nc.vector.memset(osb[:, 0:8], 0)
            for b, chunks, ename in ([] if cfg.get("no_adds") else add_plan):
                c0 = chunks[0]
                nch = len(chunks)
                j0 = 3 * b + c0
                if nch > 1:
                    t_b = bass.AP(
                        ts12.tensor, j0 * F + 47,
                        [[TP, 128], [F, nch], [-1, F], [0, 8]],
                    )
                    v_b = bass.AP(
                        vt12.tensor, j0 * 9,
                        [[VP, 128], [9, nch], [0, F], [1, 8]],
                    )
                    o_ap = bass.AP(
                        osb.tensor, j0 * L,
                        [[B * 3 * L, 128], [L, nch], [8, F], [1, 8]],
                    )
                else:
                    t_b = bass.AP(
                        ts12.tensor, j0 * F + 47, [[TP, 128], [-1, F], [0, 8]]
                    )
                    v_b = bass.AP(
                        vt12.tensor, j0 * 9, [[VP, 128], [0, F], [1, 8]]
                    )
                    o_ap = bass.AP(
                        osb.tensor, j0 * L, [[B * 3 * L, 128], [8, F], [1, 8]]
                    )
                eng(ename).tensor_add(o_ap, t_b, v_b)
            if cfg.get("no_store"):
                return
            st_src = bass.AP(osb.tensor, 0, [[B * 3 * L, 128], [1, NJ * L]])
            st_dst = bass.AP(out, 0, [[NJ * L, 128], [1, NJ * L]])
            nc.sync.dma_start(st_dst, st_src)

        tc.For_i_pipelined(
            [stage_load, stage_mm, stage_skew, stage_out],
            0,
            trip,
            unroll=unroll,
        )

    nc.compile()
    return nc


def _get_program():
    if "nc" not in _prog_cache:
        _prog_cache["nc"] = build_program()
    return _prog_cache["nc"]


def make_core_inputs(q, r_voice, e_past, e_future):
    q = np.asarray(q, dtype=np.float32)
    qb = q.reshape(B, H, L, E)
    in_maps = []
    for h in range(NCORES):
        w = np.zeros((48, 384), np.float32)
        gfull = np.empty((E, 95), np.float32)
        gfull[:, 0:47] = e_future[1:48, :, h][::-1].T
        gfull[:, 47:95] = e_past[:, :, h].T
        u = r_voice[:, :, :, h].reshape(DI * DO, E).T  # (E, 64)
        for c in range(3):
            blk = np.zeros((E, 128), np.float32)
            blk[:, 0:63] = gfull[:, 16 * c:16 * c + 63]
            blk[:, 64:128] = u
            w[c * 16:(c + 1) * 16, 128 * c:128 * (c + 1)] = blk
        qh = qb[:, h]  # (B, L, E)
        qt = qh.reshape(B, 3, 128, E).transpose(1, 3, 0, 2).reshape(48, 512)
        in_maps.append(
            {
                "winp": w.astype(ml_dtypes.bfloat16),
                "qinp": np.ascontiguousarray(qt).astype(ml_dtypes.bfloat16),
                "mk": MSK,
            }
        )
    return in_maps


def unshard(out_h):
    """out_h: [128, NJ*L] device layout -> [B, L, L] float32."""
    arr = np.asarray(out_h).astype(np.float32).reshape(128, B, 3, L)
    return arr.transpose(1, 2, 0, 3).reshape(B, L, L)


def kernel(q, flipped_masks, r_voice, e_past, e_future):
    q = np.asarray(q, dtype=np.float32)
    r_voice = np.asarray(r_voice, dtype=np.float32)
    e_past = np.asarray(e_past, dtype=np.float32)
    e_future = np.asarray(e_future, dtype=np.float32)

    nc = _get_program()
    in_maps = make_core_inputs(q, r_voice, e_past, e_future)
    res = run_bass_kernel_spmd(nc, in_maps, core_ids=list(range(NCORES)))

    out = np.empty((B * H, L, L), dtype=np.float32)
    for h in range(NCORES):
        out_b = unshard(res.results[h]["out"])
        for b in range(B):
            out[b * H + h] = out_b[b]
    return out
